# revision 1
# baseline (speedup 1.0000x reference)
"""Trainium2 Bass kernel for batched pairwise-distance + group-min + mean.

Computes, for x1 [8, 2048, 1024] f32 and x2 [8, 1152, 1024] f32:
    d[b, m, n] = ||x1[b,m] - x2[b,n]||^2           [8, 2048, 1152]
    out = mean over groups-of-9 minima of d (reshape [B, -1, 9].min(-1).mean())

Strategy: data-parallel over batch B=8 across the 8 NeuronCores. Each core:
  - cast-DMAs its x1/x2 shard to bf16 in SBUF, xbar-transpose-DMAs them to
    [d, n] layout,
  - computes cross[m, n] = x1 @ x2.T via bf16 matmuls accumulated in PSUM,
    with an extra K=1 matmul appending -0.5*||x2[n]||^2 per column,
  - group-MAX of (cross - 0.5*sq2) over 9 consecutive n on the vector engine
    (min of d is sq1[m] - 2 * that max; sq1 is constant within a group),
  - accumulates per-partition sums; host combines:
        sum_d_min = 128 * sum(sq1) - 2 * sum(group_max_sums)
"""
import os
import sys

for _p in ("/opt/trn_rl_repo",):
    if os.path.isdir(_p) and _p not in sys.path:
        sys.path.append(_p)

import numpy as np

B = 8
N1, D, N2 = 2048, 1024, 1152
GROUP = 9
MT, KT = N1 // 128, D // 128          # 16 m-tiles, 8 k-chunks
NG = N2 // GROUP                       # 128 groups per m-row
# psum free-dim chunks, each within one 2 KiB psum bank
CHUNKS = ((0, 512), (512, 512), (1024, 128))

_CACHE = {}


def _build():
    """Build + compile the per-core Bass program once per process."""
    from concourse import bacc, tile, mybir

    F32 = mybir.dt.float32
    BF = mybir.dt.bfloat16
    AX = mybir.AxisListType
    AF = mybir.ActivationFunctionType

    nc = bacc.Bacc("TRN2", target_bir_lowering=False, debug=False, num_devices=B,
                   dynamic_dma_scratch_size=65536)
    x1_d = nc.dram_tensor("x1", [N1, D], F32, kind="ExternalInput")
    x2_d = nc.dram_tensor("x2", [N2, D], F32, kind="ExternalInput")
    y_gm = nc.dram_tensor("y_gm", [128, MT], F32, kind="ExternalOutput")
    y_sq1 = nc.dram_tensor("y_sq1", [128, MT], F32, kind="ExternalOutput")

    with tile.TileContext(nc) as tc:
        with tc.tile_pool(name="big", bufs=1) as big, \
             tc.tile_pool(name="src", bufs=12) as srcp, \
             tc.tile_pool(name="work", bufs=2) as workp, \
             tc.tile_pool(name="ps", bufs=2, space="PSUM") as psp:

            X1T = big.tile([128, KT, N1], BF)       # x1 shard, transposed
            X2T = big.tile([128, KT, N2], BF)       # x2 shard, transposed
            X2Tsq = big.tile([128, KT, N2], BF)     # elementwise squares
            gm_out = big.tile([128, MT], F32)
            sq1_out = big.tile([128, MT], F32)
            row0f = big.tile([1, N2], F32)          # -0.5 * ||x2[n]||^2
            row0b = big.tile([128, N2], F32)        # broadcast to all partitions
            ones_w = big.tile([128, 1], BF)

            nc.vector.memset(ones_w[:], 1.0)

            # ---- loads: x1 group 0 first (mains need X1T t0 earliest), then
            #      x2 (4+5 tiles), then rest of x1. One SWDGE queue, in order.
            x1_view = x1_d.ap().rearrange("(g tl p) d -> g p tl d", g=4, tl=4, p=128)
            x1srcs = []
            x1src0 = srcp.tile([128, 4, D], BF, tag="x1src", bufs=4, name="x1src0")
            nc.gpsimd.dma_start(out=x1src0[:], in_=x1_view[0])
            x1srcs.append(x1src0)

            x2a = srcp.tile([128, 4, D], BF, tag="x2a", bufs=1, name="x2a")
            nc.gpsimd.dma_start(
                out=x2a[:],
                in_=x2_d.ap()[0:512, :].rearrange("(tl p) d -> p tl d", p=128))
            x2b = srcp.tile([128, 5, D], BF, tag="x2b", bufs=1, name="x2b")
            nc.gpsimd.dma_start(
                out=x2b[:],
                in_=x2_d.ap()[512:N2, :].rearrange("(tl p) d -> p tl d", p=128))

            for g in range(1, 4):
                x1src = srcp.tile([128, 4, D], BF, tag="x1src", bufs=4,
                                  name=f"x1src{g}")
                nc.gpsimd.dma_start(out=x1src[:], in_=x1_view[g])
                x1srcs.append(x1src)

            # ---- transposes: x1 t0-7 on ACT HWDGE, x2 + x1 t8-15 on SP.
            #      Square each transposed x2 slice immediately (DVE). ----
            for tl in range(4):
                nc.scalar.dma_start(out=X1T[:, :, tl * 128:(tl + 1) * 128],
                                    in_=x1src0[:, tl, :], transpose=True)

            def x2_transpose(t, src, tl):
                nc.sync.dma_start(out=X2T[:, :, t * 128:(t + 1) * 128],
                                  in_=src[:, tl, :], transpose=True)
                nc.vector.tensor_mul(X2Tsq[:, :, t * 128:(t + 1) * 128],
                                     X2T[:, :, t * 128:(t + 1) * 128],
                                     X2T[:, :, t * 128:(t + 1) * 128])

            for tl in range(4):
                x2_transpose(tl, x2a, tl)
            for tl in range(5):
                x2_transpose(4 + tl, x2b, tl)

            for g in range(1, 4):
                for tl in range(4):
                    t = 4 * g + tl
                    dma_eng = nc.scalar if g < 1 else nc.sync
                    dma_eng.dma_start(out=X1T[:, :, t * 128:(t + 1) * 128],
                                      in_=x1srcs[g][:, tl, :], transpose=True)

            # ---- main loop. PE order: m0 mains, sq2 ones-matmul, m1.. mains.
            #      sq2 row is folded in on DVE (no PE appends): the group
            #      statistic is max(cross - 0.5*sq2) over each 9-column group.
            def mains(m):
                ps = psp.tile([128, N2], F32, tag="mm", name=f"ps{m}")
                for k in range(KT):
                    for c, (off, w) in enumerate(CHUNKS):
                        nc.tensor.matmul(ps[:, off:off + w],
                                         lhsT=X1T[:, k, m * 128:(m + 1) * 128],
                                         rhs=X2T[:, k, off:off + w],
                                         start=(k == 0), stop=(k == KT - 1))
                return ps

            def epilogue(m, ps):
                e_bf = workp.tile([128, N2], BF, tag="ebuf", name=f"ebuf{m}")
                nc.vector.tensor_add(e_bf[:], ps[:], row0b[:])
                gmax = workp.tile([128, NG], F32, tag="gmax", name=f"gmax{m}")
                nc.vector.tensor_reduce(
                    out=gmax[:], in_=e_bf[:].rearrange("p (g n) -> p g n", n=GROUP),
                    axis=AX.X, op=mybir.AluOpType.max)
                nc.vector.reduce_sum(out=gm_out[:, m:m + 1], in_=gmax[:], axis=AX.X)

            ps0 = mains(0)

            # sq2 row: column-sum of squares via ones-matmul, then -0.5x and
            # broadcast to all partitions (GPSIMD) for the DVE epilogue add
            ps_row = psp.tile([1, N2], F32, tag="mm")
            for k in range(KT):
                for c, (off, w) in enumerate(CHUNKS):
                    nc.tensor.matmul(ps_row[:, off:off + w], lhsT=ones_w[:],
                                     rhs=X2Tsq[:, k, off:off + w],
                                     start=(k == 0), stop=(k == KT - 1))
            nc.vector.tensor_scalar_mul(row0f[:], ps_row[:], -0.5)
            nc.gpsimd.partition_broadcast(row0b[:], row0f[:])

            epilogue(0, ps0)
            for m in range(1, MT):
                ps = mains(m)
                epilogue(m, ps)

            # ---- sq1 via ACT square-accumulate (feeds only the output) ----
            for g in range(4):
                for tl in range(4):
                    t = 4 * g + tl
                    act_sc = workp.tile([128, D], BF, tag="actsc", name=f"actsc{t}")
                    nc.scalar.activation(out=act_sc[:], in_=x1srcs[g][:, tl, :],
                                         func=AF.Square,
                                         accum_out=sq1_out[:, t:t + 1])

            nc.sync.dma_start(out=y_gm.ap(), in_=gm_out[:])
            nc.sync.dma_start(out=y_sq1.ap(), in_=sq1_out[:])

    nc.compile()
    return nc


def get_nc():
    if "nc" not in _CACHE:
        _CACHE["nc"] = _build()
    return _CACHE["nc"]


def kernel(x1, x2):
    from concourse import bass_utils

    x1 = np.asarray(x1, dtype=np.float32)
    x2 = np.asarray(x2, dtype=np.float32)
    assert x1.shape == (B, N1, D) and x2.shape == (B, N2, D)

    nc = get_nc()
    # shard: batch b -> core b
    in_maps = [{"x1": x1[b], "x2": x2[b]} for b in range(B)]
    res = bass_utils.run_bass_kernel_spmd(nc, in_maps, core_ids=list(range(B)))

    # unshard: combine per-core partial sums (the all-reduce of the mean)
    total = 0.0
    for b in range(B):
        gm = np.asarray(res.results[b]["y_gm"], dtype=np.float64)
        sq1 = np.asarray(res.results[b]["y_sq1"], dtype=np.float64)
        total += NG * sq1.sum() - 2.0 * gm.sum()
    mean = total / (B * N1 * NG)
    return np.asarray(mean, dtype=np.float32)



# revision 3
# speedup vs baseline: 1.6008x; 1.6008x over previous
"""Trainium2 Bass kernel for batched pairwise-distance + group-min + mean.

Computes, for x1 [8, 2048, 1024] f32 and x2 [8, 1152, 1024] f32:
    d[b, m, n] = ||x1[b,m] - x2[b,n]||^2           [8, 2048, 1152]
    out = mean over groups-of-9 minima of d (reshape [B, -1, 9].min(-1).mean())

Strategy: data-parallel over batch B=8 across the 8 NeuronCores. Each core:
  - cast-DMAs x1/x2 to fp8e4 (e4m3) in SBUF,
  - transposes them with the xbar DMA by viewing adjacent fp8 pairs as one
    bf16 element (halves transpose traffic); the pair-interleaved transposed
    layout is exactly the operand format of DoubleRow fp8 matmuls,
  - computes cross[m, n] = x1 @ x2.T via fp8 DoubleRow matmuls (2 k-rows per
    partition per instruction) accumulated in PSUM,
  - appends -0.5*||x2[n]||^2 per column with a K=9 bf16 one-hot matmul
    (sq2 via ACT square-accumulate, PE-transposed to a [9,128] row tile),
  - group-MAX of (cross - 0.5*sq2) over 9 consecutive n on the vector engine
    (min of d is sq1[m] - 2 * that max; sq1 is constant within a group),
  - sq1 via ACT square-accumulate; host combines:
        sum_d_min = 128 * sum(sq1) - 2 * sum(group_max_sums)
"""
import os
import sys

for _p in ("/opt/trn_rl_repo",):
    if os.path.isdir(_p) and _p not in sys.path:
        sys.path.append(_p)

import numpy as np

B = 8
N1, D, N2 = 2048, 1024, 1152
GROUP = 9
MT = N1 // 128                        # 16 m-tiles
KK = 4                                 # DoubleRow k-steps (4 x 256 = 1024)
TLX = N2 // 128                        # 9 x2 column tiles
NG = N2 // GROUP                       # 128 groups per m-row

_CACHE = {}


def _build():
    """Build + compile the per-core Bass program once per process."""
    from concourse import bacc, tile, mybir
    from concourse.masks import make_identity

    F32 = mybir.dt.float32
    BF = mybir.dt.bfloat16
    F8 = mybir.dt.float8e4
    AX = mybir.AxisListType
    AF = mybir.ActivationFunctionType
    DR = mybir.MatmulPerfMode.DoubleRow

    nc = bacc.Bacc("TRN2", target_bir_lowering=False, debug=False, num_devices=B,
                   dynamic_dma_scratch_size=65536)
    x1_d = nc.dram_tensor("x1", [N1, D], F32, kind="ExternalInput")
    x2_d = nc.dram_tensor("x2", [N2, D], F32, kind="ExternalInput")
    y_gm = nc.dram_tensor("y_gm", [128, MT], F32, kind="ExternalOutput")
    y_sq1 = nc.dram_tensor("y_sq1", [128, MT], F32, kind="ExternalOutput")

    with tile.TileContext(nc) as tc:
        with tc.tile_pool(name="big", bufs=1) as big, \
             tc.tile_pool(name="work", bufs=2) as workp, \
             tc.tile_pool(name="ps", bufs=2, space="PSUM") as psp, \
             tc.tile_pool(name="psaux", bufs=1, space="PSUM") as psaux:

            x1p8 = big.tile([128, MT, D], F8)       # x1, m-major, fp8
            x2p8 = big.tile([128, TLX, D], F8)      # x2, n-major, fp8
            X1T = big.tile([128, MT, KK, 128], BF)  # pair-transposed x1
            X2T = big.tile([128, TLX, KK, 128], BF)
            ident = big.tile([128, 128], BF)
            onehot = big.tile([TLX, N2], BF)        # row k ones in block k
            sq2c = big.tile([128, TLX], F32)
            sq2cb = big.tile([128, TLX], BF)
            sq2T = big.tile([TLX, 128], BF)
            sq1_out = big.tile([128, MT], F32)
            gm_all = big.tile([128, MT, NG], BF)
            y_gm_t = big.tile([128, MT], F32)

            # ---- constants (Pool engine, overlapped with loads) ----
            make_identity(nc, ident)
            nc.gpsimd.memset(onehot[:], 1.0)
            nc.gpsimd.affine_select(out=onehot[:], in_=onehot[:],
                                    compare_op=mybir.AluOpType.is_ge, fill=0.0,
                                    base=0, pattern=[[1, N2]],
                                    channel_multiplier=-128)
            nc.gpsimd.affine_select(out=onehot[:], in_=onehot[:],
                                    compare_op=mybir.AluOpType.is_ge, fill=0.0,
                                    base=127, pattern=[[-1, N2]],
                                    channel_multiplier=128)

            # ---- loads (SWDGE, f32 -> fp8 cast). x1 group 0 first so its
            #      transposes can slot in right after x2's. ----
            x1_view = x1_d.ap().rearrange("(g s p) d -> g p s d", g=4, p=128)
            nc.gpsimd.dma_start(out=x1p8[:, 0:4, :], in_=x1_view[0])
            x2_view = x2_d.ap().rearrange("(g s p) d -> g p s d", g=3, p=128)
            for g in range(3):
                nc.gpsimd.dma_start(out=x2p8[:, 3 * g:3 * g + 3, :], in_=x2_view[g])
            for g in range(1, 4):
                nc.gpsimd.dma_start(out=x1p8[:, 4 * g:4 * g + 4, :], in_=x1_view[g])

            # ---- transposes (HWDGE xbar on bf16 pair view) + ACT squares ----
            for s in range(TLX):
                nc.sync.dma_start(out=X2T[:, s, :, :],
                                  in_=x2p8[:, s, :].bitcast(BF), transpose=True)
                nc.scalar.activation(out=workp.tile([128, D], BF, tag="scr",
                                                    name=f"scr2_{s}")[:],
                                     in_=x2p8[:, s, :], func=AF.Square,
                                     accum_out=sq2c[:, s:s + 1])
            for s in range(MT):
                eng = nc.sync if s % 2 == 0 else nc.scalar
                eng.dma_start(out=X1T[:, s, :, :],
                              in_=x1p8[:, s, :].bitcast(BF), transpose=True)

            # ---- sq2 row: scale by -0.5, PE-transpose to [9, 128] ----
            nc.vector.tensor_scalar_mul(sq2cb[:], sq2c[:], -0.5)
            ps_t = psaux.tile([TLX, 128], BF, tag="tr", name="sq2T_ps")
            nc.tensor.matmul(ps_t[:], lhsT=sq2cb[:], rhs=ident[:],
                             is_transpose=True)
            nc.vector.tensor_copy(sq2T[:], ps_t[:])

            # ---- PE warmup: junk matmuls to ramp the pstate clock before
            #      the real mains issue (they only touch a scratch bank) ----
            ps_w = psaux.tile([128, 512], F32, tag="warm", name="warm")
            warm_src = big.tile([128, 512], BF)
            nc.gpsimd.memset(warm_src[:], 0.0)
            for i in range(8):
                nc.tensor.matmul(ps_w[:], lhsT=ident[:], rhs=warm_src[:],
                                 start=True, stop=True, skip_group_check=True)

            # ---- main loop: fp8 DoubleRow mains + bf16 one-hot sq2 append,
            #      then DVE group-max epilogue ----
            def mains(t):
                ps = psp.tile([128, N2], F32, tag="mm", name=f"ps{t}")
                lhs_all = X1T[:, t, :, :].bitcast(F8)      # [128, KK, 256]
                for tlx in range(TLX):
                    rhs_all = X2T[:, tlx, :, :].bitcast(F8)
                    for kk in range(KK):
                        lhsT = lhs_all[:, kk, :].rearrange("p (m i) -> p i m", i=2)
                        rhs = rhs_all[:, kk, :].rearrange("p (n i) -> p i n", i=2)
                        nc.tensor.matmul(ps[:, tlx * 128:(tlx + 1) * 128],
                                         lhsT=lhsT, rhs=rhs,
                                         start=(kk == 0 and tlx in (0, 4, 8)),
                                         stop=False, perf_mode=DR)
                for tlx in range(TLX):
                    nc.tensor.matmul(ps[:, tlx * 128:(tlx + 1) * 128],
                                     lhsT=onehot[:, tlx * 128:(tlx + 1) * 128],
                                     rhs=sq2T[:],
                                     start=False, stop=(tlx in (3, 7, 8)))
                return ps

            def epilogue(t, ps):
                nc.vector.tensor_reduce(
                    out=gm_all[:, t, :],
                    in_=ps[:].rearrange("p (g n) -> p g n", n=GROUP),
                    axis=AX.X, op=mybir.AluOpType.max)

            for t in range(MT):
                ps = mains(t)
                epilogue(t, ps)

            # ---- sq1 via ACT square-accumulate on fp8 x1 ----
            for s in range(MT):
                nc.scalar.activation(out=workp.tile([128, D], BF, tag="scr",
                                                    name=f"scr1_{s}")[:],
                                     in_=x1p8[:, s, :], func=AF.Square,
                                     accum_out=sq1_out[:, s:s + 1])

            # ---- final per-m-tile sums of group maxima ----
            nc.vector.tensor_reduce(out=y_gm_t[:],
                                    in_=gm_all[:].rearrange("p t g -> p t g"),
                                    axis=AX.X, op=mybir.AluOpType.add)
            nc.sync.dma_start(out=y_gm.ap(), in_=y_gm_t[:])
            nc.sync.dma_start(out=y_sq1.ap(), in_=sq1_out[:])

    nc.compile()
    return nc


def get_nc():
    if "nc" not in _CACHE:
        _CACHE["nc"] = _build()
    return _CACHE["nc"]


def kernel(x1, x2):
    from concourse import bass_utils

    x1 = np.asarray(x1, dtype=np.float32)
    x2 = np.asarray(x2, dtype=np.float32)
    assert x1.shape == (B, N1, D) and x2.shape == (B, N2, D)

    nc = get_nc()
    # shard: batch b -> core b
    in_maps = [{"x1": x1[b], "x2": x2[b]} for b in range(B)]
    res = bass_utils.run_bass_kernel_spmd(nc, in_maps, core_ids=list(range(B)))

    # unshard: combine per-core partial sums (the all-reduce of the mean)
    total = 0.0
    for b in range(B):
        gm = np.asarray(res.results[b]["y_gm"], dtype=np.float64)
        sq1 = np.asarray(res.results[b]["y_sq1"], dtype=np.float64)
        total += NG * sq1.sum() - 2.0 * gm.sum()
    mean = total / (B * N1 * NG)
    return np.asarray(mean, dtype=np.float32)
